# revision 20
# baseline (speedup 1.0000x reference)
"""Trainium2 Bass kernel for nn_MoELayer_26242250179174.

MoE layer: B=256 samples x 63 agent-tokens, router top-2 of 8 experts,
expert MLP 128 -> 256(relu) -> 160, gate-weighted combine.

Sharding: data-parallel over batch across 8 NeuronCores (32 samples/core).
No collectives. Each core computes its output slice independently.

V0 design (dense): all 8 experts computed for all tokens; combine with the
dense gate matrix via per-tile PSUM matmuls + ACT scale + DVE accumulate.

Per-core layout (SBUF partition bases must be 0/32/64/96):
  - token tiles of 128 partitions = 2 samples x 64 rows; rows 0..62 / 64..126
    hold agents 1..63, rows 63/127 are zeroed pads. 16 tiles (TOK2=2048 cols).
  - x loaded token-major [128, 16*128], PE-transposed to xT [128(D), 2048].
  - router fp32 on-chip (near-tie in probs: gap 6e-7 -> fp32 mandatory);
    top-2 picked on unnormalized exp via Max8 + match_replace.
  - layer1 feature-major: h_T[c] = w1[e,:,c].T @ xT   (2 chunks of 128)
  - layer2 token-major:   y[tile] = h_T[:, tile].T @ w2[e]  (PSUM, 2 k-chunks)
  - combine: acc[tile] += gate[token,e] * y  (ACT copy-with-scale + DVE add)
"""

import numpy as np

B, N, D, E = 256, 64, 128, 8
H, O = 256, 160            # expert hidden, out features (T*2)
M = 8                      # cores
BS = B // M                # 32 samples per core
AG = N - 1                 # 63 agent tokens per sample
NT = BS // 2               # 16 token tiles per core
TOK2 = NT * 128            # 2048 padded token columns

_CACHE = {}


def _build():
    import concourse.bass as bass
    import concourse.tile as tile
    import concourse.mybir as mybir
    from contextlib import ExitStack

    f32 = mybir.dt.float32
    AF = mybir.ActivationFunctionType
    ALU = mybir.AluOpType
    ts = bass.ts

    nc = bass.Bass("TRN2", target_bir_lowering=False, debug=False)

    # host-packed x: tile t rows 0..62 = sample 2t agents 1..63, rows
    # 64..126 = sample 2t+1 agents 1..63, rows 63/127 zero pads
    x_d = nc.dram_tensor("xp", [NT, 128, D], f32, kind="ExternalInput")
    rw_d = nc.dram_tensor("router_w", [D, E], f32, kind="ExternalInput")
    w1_d = nc.dram_tensor("w1", [E, D, H], f32, kind="ExternalInput")
    b1_d = nc.dram_tensor("b1", [E, H], f32, kind="ExternalInput")
    w2_d = nc.dram_tensor("w2", [E, H, O], f32, kind="ExternalInput")
    id_d = nc.dram_tensor("ident", [128, 128], f32, kind="ExternalInput")
    s2_d = nc.dram_tensor("s2", [128, 2], f32, kind="ExternalInput")
    sel_d = nc.dram_tensor("sel", [BS, TOK2], f32, kind="ExternalInput")
    # padded output; host drops pad rows 63/127 and interleaves samples
    out_d = nc.dram_tensor("out", [NT, 128, O], f32, kind="ExternalOutput")

    with tile.TileContext(nc) as tc, ExitStack() as ctx:
        const = ctx.enter_context(tc.tile_pool(name="const", bufs=1))
        sb = ctx.enter_context(tc.tile_pool(name="sb", bufs=1))

        # ---- constants / weights in SBUF ----
        id_sb = const.tile([128, 128], f32)
        nc.sync.dma_start(id_sb[:], id_d.ap())
        rw_sb = const.tile([128, E], f32)
        nc.sync.dma_start(rw_sb[:], rw_d.ap())
        s2_sb = const.tile([128, 2], f32)
        nc.sync.dma_start(s2_sb[:], s2_d.ap())
        sel_sb = const.tile([BS, TOK2], f32)
        nc.sync.dma_start(sel_sb[:], sel_d.ap())
        # w1 as [D=128, E*H]
        w1_sb = const.tile([128, E * H], f32)
        nc.sync.dma_start(
            w1_sb[:].rearrange("d (e h) -> d e h", e=E),
            w1_d.ap().rearrange("e d h -> d e h"),
        )
        # b1 as [128, E*2] (chunk c of expert e in column e*2+c)
        b1_sb = const.tile([128, E * 2], f32)
        nc.sync.dma_start(
            b1_sb[:].rearrange("p (e c) -> p e c", e=E),
            b1_d.ap().rearrange("e (c p) -> p e c", p=128),
        )
        # w2 chunk c as [128, E*O]
        w2c_sb = [const.tile([128, E * O], f32, tag=f"w2c{c}", name=f"w2c{c}")
                  for c in range(2)]
        for c in range(2):
            nc.sync.dma_start(
                w2c_sb[c][:].rearrange("h (e o) -> h e o", e=E),
                w2_d.ap()[:, c * 128:(c + 1) * 128, :].rearrange("e h o -> h e o"),
            )

        # ---- load x token-major: [128, 16*128] (host-packed, pads zero) ----
        x_sb = sb.tile([128, TOK2], f32)
        nc.sync.dma_start(
            x_sb[:].rearrange("p (t d) -> p t d", d=D),
            x_d.ap().rearrange("t p d -> p t d"),
        )

        # ---- transpose to xT [128(D), 2048] + pooled accumulation ----
        xT_sb = sb.tile([128, TOK2], f32)
        phase_a = ExitStack()
        pool_ps = phase_a.enter_context(
            tc.tile_pool(name="pool_ps", bufs=1, space="PSUM"))
        tr_ps = phase_a.enter_context(
            tc.tile_pool(name="tr_ps", bufs=2, space="PSUM"))
        pooled_ps = pool_ps.tile([128, BS], f32)
        for t in range(NT):
            xt_ps = tr_ps.tile([128, 128], f32)
            nc.tensor.transpose(
                xt_ps[:], x_sb[:, ts(t, 128)], id_sb[:])
            if t % 2 == 0:
                nc.vector.tensor_copy(xT_sb[:, ts(t, 128)], xt_ps[:])
            else:
                nc.scalar.copy(xT_sb[:, ts(t, 128)], xt_ps[:])
            # pooled_T[:, 2t:2t+2] = x_tile.T @ s2   (sum over 63 agents)
            nc.tensor.matmul(
                pooled_ps[:, 2 * t:2 * t + 2], x_sb[:, ts(t, 128)], s2_sb[:],
                start=True, stop=True)

        # ---- router (all fp32, tiny) ----
        pooled_sb = sb.tile([128, BS], f32)
        nc.vector.tensor_scalar_mul(pooled_sb[:], pooled_ps[:], 1.0 / AG)
        logit_ps = pool_ps.tile([BS, E], f32)
        nc.tensor.matmul(logit_ps[:], pooled_sb[:], rw_sb[:, 0:E],
                         start=True, stop=True)
        logits = sb.tile([BS, E], f32)
        nc.vector.tensor_copy(logits[:], logit_ps[:])
        negm = sb.tile([BS, 1], f32)
        nc.vector.tensor_reduce(negm[:], logits[:], axis=mybir.AxisListType.X,
                                op=ALU.max, negate=True)
        ex = sb.tile([BS, E], f32)
        nc.scalar.activation(ex[:], logits[:], AF.Exp, bias=negm[:, 0:1])
        ssum = sb.tile([BS, 1], f32)
        nc.vector.tensor_reduce(ssum[:], ex[:], axis=mybir.AxisListType.X,
                                op=ALU.add)
        rcp = sb.tile([BS, 1], f32)
        nc.vector.reciprocal(rcp[:], ssum[:])
        # top-2 on unnormalized exp values (all > 0)
        mx8 = sb.tile([BS, 8], f32)
        nc.vector.max(out=mx8[:], in_=ex[:])
        nc.vector.memset(mx8[:, 2:8], 0.0)
        zap = sb.tile([BS, E], f32)
        nc.vector.match_replace(out=zap[:], in_to_replace=mx8[:],
                                in_values=ex[:], imm_value=0.0)
        sel_g = sb.tile([BS, E], f32)
        nc.vector.tensor_sub(sel_g[:], ex[:], zap[:])
        gates = sb.tile([BS, E], f32)
        nc.vector.tensor_scalar_mul(gates[:], sel_g[:], rcp[:, 0:1])

        # ---- expand gates to per-token tiles gt [128, 16*8] ----
        gt_sb = sb.tile([128, NT * E], f32)
        for t in range(NT):
            gt_ps = tr_ps.tile([128, E], f32, tag="gtps")
            nc.tensor.matmul(gt_ps[:], sel_sb[:, ts(t, 128)], gates[:],
                             start=True, stop=True)
            nc.vector.tensor_copy(gt_sb[:, ts(t, E)], gt_ps[:])
        phase_a.close()

        # ---- experts ----
        h_pool = ctx.enter_context(tc.tile_pool(name="h", bufs=2))
        h_ps_pool = ctx.enter_context(
            tc.tile_pool(name="h_ps", bufs=3, space="PSUM"))
        y_ps_pool = ctx.enter_context(
            tc.tile_pool(name="y_ps", bufs=4, space="PSUM"))
        tmp_pool = ctx.enter_context(tc.tile_pool(name="tmp", bufs=6))
        acc_sb = sb.tile([128, NT * O], f32)

        NQ = 4            # token quarters for layer1 (512 cols = 1 PSUM bank)
        QW = TOK2 // NQ   # 512

        for e in range(E):
            # layer1: h_T[c] = relu(w1[e,:,c*128:+128].T @ xT + b1)
            h_sb = [h_pool.tile([128, TOK2], f32, tag=f"h{c}", name=f"h{c}")
                    for c in range(2)]
            for c in range(2):
                b1col = b1_sb[:, (e * 2 + c):(e * 2 + c) + 1]
                for q in range(NQ):
                    h_ps = h_ps_pool.tile([128, QW], f32)
                    nc.tensor.matmul(
                        h_ps[:], w1_sb[:, ts(e * 2 + c, 128)],
                        xT_sb[:, ts(q, QW)], start=True, stop=True)
                    if (c + q) % 2 == 0:
                        nc.scalar.activation(h_sb[c][:, ts(q, QW)], h_ps[:],
                                             AF.Relu, bias=b1col)
                    else:
                        nc.vector.tensor_scalar(
                            h_sb[c][:, ts(q, QW)], h_ps[:], b1col, 0.0,
                            op0=ALU.add, op1=ALU.max)
            # layer2 + combine per token tile
            for t in range(NT):
                y_ps = y_ps_pool.tile([128, O], f32)
                nc.tensor.matmul(y_ps[:], h_sb[0][:, ts(t, 128)],
                                 w2c_sb[0][:, ts(e, O)], start=True, stop=False)
                nc.tensor.matmul(y_ps[:], h_sb[1][:, ts(t, 128)],
                                 w2c_sb[1][:, ts(e, O)], start=False, stop=True)
                g_col = gt_sb[:, (t * E + e):(t * E + e) + 1]
                if e == 0:
                    nc.scalar.activation(acc_sb[:, ts(t, O)], y_ps[:],
                                         AF.Copy, scale=g_col)
                else:
                    tmp = tmp_pool.tile([128, O], f32)
                    # balance scale+add across ACT / DVE / GpSimd
                    if (t + e) % 2 == 0:
                        nc.scalar.activation(tmp[:], y_ps[:], AF.Copy,
                                             scale=g_col)
                        nc.vector.tensor_add(acc_sb[:, ts(t, O)],
                                             acc_sb[:, ts(t, O)], tmp[:])
                    else:
                        nc.vector.tensor_scalar_mul(tmp[:], y_ps[:], g_col)
                        nc.gpsimd.tensor_add(acc_sb[:, ts(t, O)],
                                             acc_sb[:, ts(t, O)], tmp[:])

        # ---- store output (padded; host strips pad rows) ----
        nc.sync.dma_start(
            out_d.ap().rearrange("t p o -> p t o"),
            acc_sb[:].rearrange("p (t o) -> p t o", o=O),
        )

    return nc


def _split_multi_waits(nc):
    """walrus on this toolchain rejects instructions with >1 sync wait
    ("Too many sync wait commands"). Hoist all but the last wait of any
    instruction onto standalone EventSemaphore waits on the same engine,
    inserted immediately before it (engine queues drain in program order,
    so semantics are preserved)."""
    import concourse.mybir as mybir

    n = 0
    for fn in nc.m.functions:
        for blk in fn.blocks:
            new_insts = []
            for inst in blk.instructions:
                si = inst.sync_info
                if si is not None and si.on_wait and len(si.on_wait) > 1:
                    for w in si.on_wait[:-1]:
                        n += 1
                        ev = mybir.InstEventSemaphore(
                            name=f"WSPLIT-{n}",
                            ins=[], outs=[],
                            engine=inst.engine,
                            sync_info=mybir.SyncInfo(on_wait=[w], on_update=[]),
                        )
                        new_insts.append(ev)
                    inst.sync_info = mybir.SyncInfo(
                        on_wait=[si.on_wait[-1]], on_update=si.on_update)
                new_insts.append(inst)
            blk.instructions = new_insts
    return n


def _get_nc(split=True):
    """split=True: walrus-compatible program (multi-waits hoisted).
    split=False: pristine program for CoreSim."""
    key = f"nc_split{split}"
    if key not in _CACHE:
        nc = _build()
        if split:
            _split_multi_waits(nc)
        _CACHE[key] = nc
    return _CACHE[key]


def _aux():
    ident = np.eye(128, dtype=np.float32)
    s2 = np.zeros((128, 2), dtype=np.float32)
    s2[0:AG, 0] = 1.0
    s2[64:64 + AG, 1] = 1.0
    sel = np.zeros((BS, TOK2), dtype=np.float32)
    for t in range(NT):
        sel[2 * t, t * 128:t * 128 + 64] = 1.0
        sel[2 * t + 1, t * 128 + 64:t * 128 + 128] = 1.0
    return ident, s2, sel


def _pack_x(xc):
    """[BS, N, D] core slice -> [NT, 128, D] padded tile layout."""
    xp = np.zeros((NT, 128, D), dtype=np.float32)
    xp[:, 0:AG, :] = xc[0::2, 1:N, :]
    xp[:, 64:64 + AG, :] = xc[1::2, 1:N, :]
    return xp


def _unpack_out(oc):
    """[NT, 128, O] padded -> [BS, AG, O]."""
    out = np.empty((BS, AG, O), dtype=np.float32)
    out[0::2] = oc[:, 0:AG, :]
    out[1::2] = oc[:, 64:64 + AG, :]
    return out


def _in_maps(x, router_w, w1, b1, w2):
    ident, s2, sel = _aux()
    maps = []
    for c in range(M):
        maps.append({
            "xp": _pack_x(x[c * BS:(c + 1) * BS]),
            "router_w": np.ascontiguousarray(router_w),
            "w1": np.ascontiguousarray(w1),
            "b1": np.ascontiguousarray(b1),
            "w2": np.ascontiguousarray(w2),
            "ident": ident,
            "s2": s2,
            "sel": sel,
        })
    return maps


def kernel(x, router_w, router_b, w1, b1, w2, b2, A, _sim=False, _trace=False):
    x = np.asarray(x, dtype=np.float32)
    router_w = np.asarray(router_w, dtype=np.float32)
    w1 = np.asarray(w1, dtype=np.float32)
    b1 = np.asarray(b1, dtype=np.float32)
    w2 = np.asarray(w2, dtype=np.float32)
    # router_b/b2 are structurally zero in this problem; the on-chip program
    # folds b1 only. Guard so a nonzero bias can't silently give wrong output.
    assert not np.any(np.asarray(router_b)), "router_b must be zero"
    assert not np.any(np.asarray(b2)), "b2 must be zero"
    assert int(A) == N

    nc = _get_nc(split=not _sim)
    maps = _in_maps(x, router_w, w1, b1, w2)

    if _sim:
        from concourse.bass_interp import CoreSim
        outs = []
        for c in range(M):
            sim = CoreSim(nc, trace=False)
            for k, v in maps[c].items():
                sim.tensor(k)[:] = v
            sim.simulate(check_with_hw=False)
            outs.append(_unpack_out(np.array(sim.tensor("out"))))
            if c == 0 and _sim == "one":
                return np.concatenate([outs[0]] * M, axis=0).reshape(
                    B, AG, O // 2, 2)
        return np.concatenate(outs, axis=0).reshape(B, AG, O // 2, 2)

    from concourse.bass_utils import run_bass_kernel_spmd
    res = run_bass_kernel_spmd(nc, maps, core_ids=list(range(M)),
                               trace=bool(_trace))
    _CACHE["last_result"] = res
    out = np.concatenate(
        [_unpack_out(res.results[c]["out"]) for c in range(M)], axis=0)
    return out.reshape(B, AG, O // 2, 2)


# revision 28
# speedup vs baseline: 1.0178x; 1.0178x over previous
"""Trainium2 Bass kernel for nn_MoELayer_26242250179174.

MoE layer: B=256 samples x 63 agent-tokens, router top-2 of 8 experts,
expert MLP 128 -> 256(relu) -> 160, gate-weighted combine.

Sharding: data-parallel over batch across 8 NeuronCores (32 samples/core).
No collectives. Each core computes its output slice independently.

V0 design (dense): all 8 experts computed for all tokens; combine with the
dense gate matrix via per-tile PSUM matmuls + ACT scale + DVE accumulate.

Per-core layout (SBUF partition bases must be 0/32/64/96):
  - token tiles of 128 partitions = 2 samples x 64 rows; rows 0..62 / 64..126
    hold agents 1..63, rows 63/127 are zeroed pads. 16 tiles (TOK2=2048 cols).
  - x loaded token-major [128, 16*128], PE-transposed to xT [128(D), 2048].
  - router fp32 on-chip (near-tie in probs: gap 6e-7 -> fp32 mandatory);
    top-2 picked on unnormalized exp via Max8 + match_replace.
  - layer1 feature-major: h_T[c] = w1[e,:,c].T @ xT   (2 chunks of 128)
  - layer2 token-major:   y[tile] = h_T[:, tile].T @ w2[e]  (PSUM, 2 k-chunks)
  - combine: acc[tile] += gate[token,e] * y  (ACT copy-with-scale + DVE add)
"""

import numpy as np

B, N, D, E = 256, 64, 128, 8
H, O = 256, 160            # expert hidden, out features (T*2)
M = 8                      # cores
BS = B // M                # 32 samples per core
AG = N - 1                 # 63 agent tokens per sample
NT = BS // 2               # 16 token tiles per core
TOK2 = NT * 128            # 2048 padded token columns

_CACHE = {}


def _build(ablate=()):
    import concourse.bass as bass
    import concourse.tile as tile
    import concourse.mybir as mybir
    from contextlib import ExitStack

    f32 = mybir.dt.float32
    AF = mybir.ActivationFunctionType
    ALU = mybir.AluOpType
    ts = bass.ts

    nc = bass.Bass("TRN2", target_bir_lowering=False, debug=False)

    # host-packed x: tile t rows 0..62 = sample 2t agents 1..63, rows
    # 64..126 = sample 2t+1 agents 1..63, rows 63/127 zero pads
    x_d = nc.dram_tensor("xp", [NT, 128, D], f32, kind="ExternalInput")
    rw_d = nc.dram_tensor("router_w", [D, E], f32, kind="ExternalInput")
    w1_d = nc.dram_tensor("w1", [E, D, H], f32, kind="ExternalInput")
    b1_d = nc.dram_tensor("b1", [E, H], f32, kind="ExternalInput")
    w2_d = nc.dram_tensor("w2", [E, H, O], f32, kind="ExternalInput")
    id_d = nc.dram_tensor("ident", [128, 128], f32, kind="ExternalInput")
    s2_d = nc.dram_tensor("s2", [128, 2], f32, kind="ExternalInput")
    sel_d = nc.dram_tensor("sel", [BS, TOK2], f32, kind="ExternalInput")
    # padded output; host drops pad rows 63/127 and interleaves samples
    out_d = nc.dram_tensor("out", [NT, 128, O], f32, kind="ExternalOutput")

    with tile.TileContext(nc) as tc, ExitStack() as ctx:
        const = ctx.enter_context(tc.tile_pool(name="const", bufs=1))
        sb = ctx.enter_context(tc.tile_pool(name="sb", bufs=1))

        # ---- constants / weights in SBUF ----
        id_sb = const.tile([128, 128], f32)
        nc.sync.dma_start(id_sb[:], id_d.ap())
        rw_sb = const.tile([128, E], f32)
        nc.sync.dma_start(rw_sb[:], rw_d.ap())
        s2_sb = const.tile([128, 2], f32)
        nc.sync.dma_start(s2_sb[:], s2_d.ap())
        sel_sb = const.tile([BS, TOK2], f32)
        nc.sync.dma_start(sel_sb[:], sel_d.ap())
        # w1 as [D=128, E*H]
        w1_sb = const.tile([128, E * H], f32)
        nc.sync.dma_start(
            w1_sb[:].rearrange("d (e h) -> d e h", e=E),
            w1_d.ap().rearrange("e d h -> d e h"),
        )
        # b1 as [128, E*2] (chunk c of expert e in column e*2+c)
        b1_sb = const.tile([128, E * 2], f32)
        nc.sync.dma_start(
            b1_sb[:].rearrange("p (e c) -> p e c", e=E),
            b1_d.ap().rearrange("e (c p) -> p e c", p=128),
        )
        # w2 chunk c as [128, E*O]
        w2c_sb = [const.tile([128, E * O], f32, tag=f"w2c{c}", name=f"w2c{c}")
                  for c in range(2)]
        for c in range(2):
            nc.sync.dma_start(
                w2c_sb[c][:].rearrange("h (e o) -> h e o", e=E),
                w2_d.ap()[:, c * 128:(c + 1) * 128, :].rearrange("e h o -> h e o"),
            )

        # ---- load x token-major: [128, 16*128] (host-packed, pads zero) ----
        x_sb = sb.tile([128, TOK2], f32)
        nc.sync.dma_start(
            x_sb[:].rearrange("p (t d) -> p t d", d=D),
            x_d.ap().rearrange("t p d -> p t d"),
        )

        # ---- transpose to xT [128(D), 2048] + pooled accumulation ----
        xT_sb = sb.tile([128, TOK2], f32)
        phase_a = ExitStack()
        pool_ps = phase_a.enter_context(
            tc.tile_pool(name="pool_ps", bufs=1, space="PSUM"))
        tr_ps = phase_a.enter_context(
            tc.tile_pool(name="tr_ps", bufs=2, space="PSUM"))
        pooled_ps = pool_ps.tile([128, BS], f32)
        for t in range(NT):
            xt_ps = tr_ps.tile([128, 128], f32)
            nc.tensor.transpose(
                xt_ps[:], x_sb[:, ts(t, 128)], id_sb[:])
            if t % 2 == 0:
                nc.vector.tensor_copy(xT_sb[:, ts(t, 128)], xt_ps[:])
            else:
                nc.scalar.copy(xT_sb[:, ts(t, 128)], xt_ps[:])
            # pooled_T[:, 2t:2t+2] = x_tile.T @ s2   (sum over 63 agents)
            nc.tensor.matmul(
                pooled_ps[:, 2 * t:2 * t + 2], x_sb[:, ts(t, 128)], s2_sb[:],
                start=True, stop=True)

        # ---- router (all fp32, tiny) ----
        pooled_sb = sb.tile([128, BS], f32)
        nc.vector.tensor_scalar_mul(pooled_sb[:], pooled_ps[:], 1.0 / AG)
        logit_ps = pool_ps.tile([BS, E], f32)
        nc.tensor.matmul(logit_ps[:], pooled_sb[:], rw_sb[:, 0:E],
                         start=True, stop=True)
        logits = sb.tile([BS, E], f32)
        nc.vector.tensor_copy(logits[:], logit_ps[:])
        negm = sb.tile([BS, 1], f32)
        nc.vector.tensor_reduce(negm[:], logits[:], axis=mybir.AxisListType.X,
                                op=ALU.max, negate=True)
        ex = sb.tile([BS, E], f32)
        nc.scalar.activation(ex[:], logits[:], AF.Exp, bias=negm[:, 0:1])
        ssum = sb.tile([BS, 1], f32)
        nc.vector.tensor_reduce(ssum[:], ex[:], axis=mybir.AxisListType.X,
                                op=ALU.add)
        rcp = sb.tile([BS, 1], f32)
        nc.vector.reciprocal(rcp[:], ssum[:])
        # top-2 on unnormalized exp values (all > 0)
        mx8 = sb.tile([BS, 8], f32)
        nc.vector.max(out=mx8[:], in_=ex[:])
        nc.vector.memset(mx8[:, 2:8], 0.0)
        zap = sb.tile([BS, E], f32)
        nc.vector.match_replace(out=zap[:], in_to_replace=mx8[:],
                                in_values=ex[:], imm_value=0.0)
        sel_g = sb.tile([BS, E], f32)
        nc.vector.tensor_sub(sel_g[:], ex[:], zap[:])
        gates = sb.tile([BS, E], f32)
        nc.vector.tensor_scalar_mul(gates[:], sel_g[:], rcp[:, 0:1])

        # ---- expand gates to per-token tiles gt [128, 16*8] ----
        gt_sb = sb.tile([128, NT * E], f32)
        for t in range(NT):
            gt_ps = tr_ps.tile([128, E], f32, tag="gtps")
            nc.tensor.matmul(gt_ps[:], sel_sb[:, ts(t, 128)], gates[:],
                             start=True, stop=True)
            nc.vector.tensor_copy(gt_sb[:, ts(t, E)], gt_ps[:])
        phase_a.close()

        # ---- experts ----
        h_pool = ctx.enter_context(tc.tile_pool(name="h", bufs=2))
        h_ps_pool = ctx.enter_context(
            tc.tile_pool(name="h_ps", bufs=2, space="PSUM"))
        y_ps_pool = ctx.enter_context(
            tc.tile_pool(name="y_ps", bufs=6, space="PSUM"))
        tmp_pool = ctx.enter_context(tc.tile_pool(name="tmp", bufs=10))
        acc_tiles = [sb.tile([128, O], f32, name=f"acc{t}", tag=f"acc{t}")
                     for t in range(NT)]

        NQ = 4            # token quarters for layer1 (512 cols = 1 PSUM bank)
        QW = TOK2 // NQ   # 512

        NE = 2 if "dense2" in ablate else E

        def layer1(e):
            # h_T[c] = relu(w1[e,:,c*128:+128].T @ xT + b1)
            h_sb = [h_pool.tile([128, TOK2], f32, tag=f"h{c}", name=f"h{c}")
                    for c in range(2)]
            for c in range(2):
                b1col = b1_sb[:, (e * 2 + c):(e * 2 + c) + 1]
                for q in range(NQ):
                    h_ps = h_ps_pool.tile([128, QW], f32, name="h_ps")
                    nc.tensor.matmul(
                        h_ps[:], w1_sb[:, ts(e * 2 + c, 128)],
                        xT_sb[:, ts(q, QW)], start=True, stop=True)
                    if (c + q) % 2 == 0:
                        nc.scalar.activation(h_sb[c][:, ts(q, QW)], h_ps[:],
                                             AF.Relu, bias=b1col)
                    else:
                        nc.vector.tensor_scalar(
                            h_sb[c][:, ts(q, QW)], h_ps[:], b1col, 0.0,
                            op0=ALU.add, op1=ALU.max)
            return h_sb

        # software pipeline: emit L1 of expert e+1 before L2 of expert e so
        # the PE runs layer1(e+1) while ACT/DVE finish relu(e) / combine(e)
        h_cur = layer1(0)
        for e in range(NE):
            h_sb = h_cur
            if e + 1 < NE:
                h_cur = layer1(e + 1)
            # layer2 + combine per token tile
            if "nol2" in ablate:
                continue
            for t in range(NT):
                if "nocombine" in ablate and e > 0:
                    break
                y_ps = y_ps_pool.tile([128, O], f32)
                nc.tensor.matmul(y_ps[:], h_sb[0][:, ts(t, 128)],
                                 w2c_sb[0][:, ts(e, O)], start=True, stop=False)
                nc.tensor.matmul(y_ps[:], h_sb[1][:, ts(t, 128)],
                                 w2c_sb[1][:, ts(e, O)], start=False, stop=True)
                g_col = gt_sb[:, (t * E + e):(t * E + e) + 1]
                if "justcopy" in ablate:
                    if e == 0:
                        nc.vector.tensor_copy(acc_tiles[t][:], y_ps[:])
                    else:
                        tmp = tmp_pool.tile([128, O], f32)
                        nc.vector.tensor_copy(tmp[:], y_ps[:])
                    continue
                if e == 0:
                    nc.scalar.activation(acc_tiles[t][:], y_ps[:],
                                         AF.Copy, scale=g_col)
                else:
                    tmp = tmp_pool.tile([128, O], f32)
                    # balance scale+add across ACT / DVE / GpSimd
                    if (t + e) % 3 == 0:
                        nc.scalar.activation(tmp[:], y_ps[:], AF.Copy,
                                             scale=g_col)
                        nc.vector.tensor_add(acc_tiles[t][:],
                                             acc_tiles[t][:], tmp[:])
                    else:
                        nc.vector.tensor_scalar_mul(tmp[:], y_ps[:], g_col)
                        nc.gpsimd.tensor_add(acc_tiles[t][:],
                                             acc_tiles[t][:], tmp[:])

        # ---- store output (padded; host strips pad rows) ----
        for t in range(NT):
            nc.sync.dma_start(out_d.ap()[t], acc_tiles[t][:])

    return nc


def _split_multi_waits(nc):
    """walrus on this toolchain rejects instructions with >1 sync wait
    ("Too many sync wait commands"). Hoist all but the last wait of any
    instruction onto standalone EventSemaphore waits on the same engine,
    inserted immediately before it (engine queues drain in program order,
    so semantics are preserved)."""
    import concourse.mybir as mybir

    n = 0
    for fn in nc.m.functions:
        for blk in fn.blocks:
            new_insts = []
            for inst in blk.instructions:
                si = inst.sync_info
                if si is not None and si.on_wait and len(si.on_wait) > 1:
                    for w in si.on_wait[:-1]:
                        n += 1
                        ev = mybir.InstEventSemaphore(
                            name=f"WSPLIT-{n}",
                            ins=[], outs=[],
                            engine=inst.engine,
                            sync_info=mybir.SyncInfo(on_wait=[w], on_update=[]),
                        )
                        new_insts.append(ev)
                    inst.sync_info = mybir.SyncInfo(
                        on_wait=[si.on_wait[-1]], on_update=si.on_update)
                new_insts.append(inst)
            blk.instructions = new_insts
    return n


def _get_nc(split=True):
    """split=True: walrus-compatible program (multi-waits hoisted).
    split=False: pristine program for CoreSim."""
    key = f"nc_split{split}"
    if key not in _CACHE:
        nc = _build()
        if split:
            _split_multi_waits(nc)
        _CACHE[key] = nc
    return _CACHE[key]


def _aux():
    ident = np.eye(128, dtype=np.float32)
    s2 = np.zeros((128, 2), dtype=np.float32)
    s2[0:AG, 0] = 1.0
    s2[64:64 + AG, 1] = 1.0
    sel = np.zeros((BS, TOK2), dtype=np.float32)
    for t in range(NT):
        sel[2 * t, t * 128:t * 128 + 64] = 1.0
        sel[2 * t + 1, t * 128 + 64:t * 128 + 128] = 1.0
    return ident, s2, sel


def _pack_x(xc):
    """[BS, N, D] core slice -> [NT, 128, D] padded tile layout."""
    xp = np.zeros((NT, 128, D), dtype=np.float32)
    xp[:, 0:AG, :] = xc[0::2, 1:N, :]
    xp[:, 64:64 + AG, :] = xc[1::2, 1:N, :]
    return xp


def _unpack_out(oc):
    """[NT, 128, O] padded -> [BS, AG, O]."""
    out = np.empty((BS, AG, O), dtype=np.float32)
    out[0::2] = oc[:, 0:AG, :]
    out[1::2] = oc[:, 64:64 + AG, :]
    return out


def _in_maps(x, router_w, w1, b1, w2):
    ident, s2, sel = _aux()
    maps = []
    for c in range(M):
        maps.append({
            "xp": _pack_x(x[c * BS:(c + 1) * BS]),
            "router_w": np.ascontiguousarray(router_w),
            "w1": np.ascontiguousarray(w1),
            "b1": np.ascontiguousarray(b1),
            "w2": np.ascontiguousarray(w2),
            "ident": ident,
            "s2": s2,
            "sel": sel,
        })
    return maps


def kernel(x, router_w, router_b, w1, b1, w2, b2, A, _sim=False, _trace=False):
    x = np.asarray(x, dtype=np.float32)
    router_w = np.asarray(router_w, dtype=np.float32)
    w1 = np.asarray(w1, dtype=np.float32)
    b1 = np.asarray(b1, dtype=np.float32)
    w2 = np.asarray(w2, dtype=np.float32)
    # router_b/b2 are structurally zero in this problem; the on-chip program
    # folds b1 only. Guard so a nonzero bias can't silently give wrong output.
    assert not np.any(np.asarray(router_b)), "router_b must be zero"
    assert not np.any(np.asarray(b2)), "b2 must be zero"
    assert int(A) == N

    nc = _get_nc(split=not _sim)
    maps = _in_maps(x, router_w, w1, b1, w2)

    if _sim:
        from concourse.bass_interp import CoreSim
        outs = []
        for c in range(M):
            sim = CoreSim(nc, trace=False)
            for k, v in maps[c].items():
                sim.tensor(k)[:] = v
            sim.simulate(check_with_hw=False)
            outs.append(_unpack_out(np.array(sim.tensor("out"))))
            if c == 0 and _sim == "one":
                return np.concatenate([outs[0]] * M, axis=0).reshape(
                    B, AG, O // 2, 2)
        return np.concatenate(outs, axis=0).reshape(B, AG, O // 2, 2)

    from concourse.bass_utils import run_bass_kernel_spmd
    res = run_bass_kernel_spmd(nc, maps, core_ids=list(range(M)),
                               trace=bool(_trace))
    _CACHE["last_result"] = res
    out = np.concatenate(
        [_unpack_out(res.results[c]["out"]) for c in range(M)], axis=0)
    return out.reshape(B, AG, O // 2, 2)
